# revision 19
# baseline (speedup 1.0000x reference)
"""MoE layer kernel for trn2, expert-parallel across 8 NeuronCores.

Sharding: core c owns routed experts [4c, 4c+4). Router + top-k + combine
weights computed on host in float64 (matching reference ranking). Tokens
are dispatched per expert on host (capacity-padded to C); device computes
unweighted SwiGLU expert outputs; the shared expert is tensor-parallel
sharded along the intermediate dim (192 rows per core). Host applies
combine weights (scatter-add) and sums the shared partials.

Memory regime: routed expert weights dominate HBM traffic, so they are
stored as float8_e3m4 (scaled by 128 with clip to +-15.5, descaled on
device / in fp32 psum) which halves DMA bytes vs bf16. Activations and
the shared expert use fp16. All three routed matmuls keep the weights as
the 128x128 stationary operand (tokens stream as the moving operand), so
gu lands directly in the [I, C] layout the down projection consumes --
no transposes anywhere. Gate and up run as separate phases so gate
compute starts as soon as the gate weights alone have arrived; shared
expert work is emitted between expert phases as pipeline filler.

DMA routing: the bulk weight stream rides the Sync HWDGE ring, chained
(depth 2 at startup, 4 in steady state) so HBM bandwidth is not smeared
across transfers needed much later; the ring then executes transfers
near emission order at full rate. Small or latency-tolerant transfers
(token blocks, shared-expert up/down weights, output writes) ride the
Scalar-sequencer ring concurrently -- keeping any slot-blocked weight
DMA off the ACT queue head, where it would stall silu evacuations and
starve the PE.
"""

import os
import sys

import numpy as np

sys.path.insert(0, "/opt/trn_rl_repo")

import ml_dtypes

import concourse.bass as bass
import concourse.mybir as mybir
import concourse.tile as tile
from concourse import bass_utils
from concourse.bass import _add_dep_helper

B, S, H = 1, 512, 2048
T = B * S
I = 1536
E = 32
K = 4
SCALE = 1.8
NCORES = 8
EL = E // NCORES          # local experts per core
IS = I // NCORES          # shared-expert intermediate shard per core
HK = H // 128             # 16 chunks of hidden
HH = HK // 2              # 8 k-chunks per gate/up weight half
IC = I // 128             # 12 chunks of intermediate
ND = 4                    # down-weight column chunks (512 wide)
WS = 128.0                # fp8 weight scale (power of 2; exact descale)
FP8_MAX = 15.5            # float8_e3m4 max normal

F16 = mybir.dt.float16
F32 = mybir.dt.float32
F8 = mybir.dt.float8e3
AF = mybir.ActivationFunctionType


def _build_program(C: int, ndev: int = NCORES, compile: bool = True):
    import concourse.bacc as bacc

    assert C <= 512
    nc = bacc.Bacc("TRN2", target_bir_lowering=False, debug=False,
                   num_devices=ndev)

    xg = nc.dram_tensor("xg", [EL, 128, HK * C], F16, kind="ExternalInput").ap()
    wg = nc.dram_tensor("wg", [EL, 2, 128, HH * I], F8, kind="ExternalInput").ap()
    wu = nc.dram_tensor("wu", [EL, 2, 128, HH * I], F8, kind="ExternalInput").ap()
    wd = nc.dram_tensor("wd", [EL, ND, 128, IC * 512], F8,
                        kind="ExternalInput").ap()
    sgw = nc.dram_tensor("sgw", [128, HK * IS], F16, kind="ExternalInput").ap()
    suw = nc.dram_tensor("suw", [128, HK * IS], F16, kind="ExternalInput").ap()
    sdw = nc.dram_tensor("sdw", [IS, H], F16, kind="ExternalInput").ap()
    xt = nc.dram_tensor("xt", [2, 128, HH * T], F16, kind="ExternalInput").ap()
    yr = nc.dram_tensor("yr", [EL, 128, HK * C], F16, kind="ExternalOutput").ap()
    ys = nc.dram_tensor("ys", [T, H], F16, kind="ExternalOutput").ap()

    # Bulk weight stream on the Sync HWDGE ring, chained at graduated
    # depth; everything else goes to the Scalar ring via
    # nc.scalar.dma_start at the call sites.
    chain = []

    def wdma(out, in_):
        inst = nc.sync.dma_start(out=out, in_=in_)
        if inst is not None:
            depth = 2 if len(chain) < 5 else 4
            if len(chain) >= depth and chain[-depth] is not None:
                _add_dep_helper(inst.ins, chain[-depth].ins, sync=True,
                                reason="dma stream ordering")
            chain.append(inst)
        return inst

    with tile.TileContext(nc) as tc:
        with (
            tc.tile_pool(name="w_sb", bufs=8) as wpool,
            tc.tile_pool(name="wdp_sb", bufs=5) as wdpool,
            tc.tile_pool(name="xg_sb", bufs=EL) as xpool,
            tc.tile_pool(name="gu_sb", bufs=2) as gupool,
            tc.tile_pool(name="silu_sb", bufs=IC + 1) as silupool,
            tc.tile_pool(name="act_sb", bufs=2) as actpool,
            tc.tile_pool(name="y_sb", bufs=3) as ypool,
            tc.tile_pool(name="sh_sb", bufs=1) as shpool,
            tc.tile_pool(name="ps_g", bufs=2, space="PSUM") as ps_g,
            tc.tile_pool(name="ps_u", bufs=2, space="PSUM") as ps_u,
            tc.tile_pool(name="ps_y", bufs=2, space="PSUM") as ps_y,
            tc.tile_pool(name="ps_s", bufs=2, space="PSUM") as ps_s,
        ):
            # ---- startup DMAs: shared gate weights + x(T) + all tokens
            sgw_sb = shpool.tile([128, HK * IS], F16, name="sgw_sb", tag="sgw")
            wdma(sgw_sb, sgw)
            xt_sb = []
            for hh in range(2):
                t_ = shpool.tile([128, HH * T], F16, name=f"xt{hh}",
                                 tag=f"xt{hh}")
                wdma(t_, xt[hh])
                xt_sb.append(t_)
            xg_sb = []
            for e in range(EL):
                t_ = xpool.tile([128, HK * C], F16, name=f"xg{e}", tag="xg")
                nc.scalar.dma_start(out=t_, in_=xg[e])
                xg_sb.append(t_)

            def xt_slice(k):
                return xt_sb[k // HH][:, bass.ts(k % HH, T)]

            suw_sb = shpool.tile([128, HK * IS], F16, name="suw_sb", tag="suw")
            sdw_sb = shpool.tile([128, 2 * H], F16, name="sdw_sb", tag="sdw")
            gus_sb = shpool.tile([128, 2 * T], F16, name="gus_sb", tag="gus")
            mchunks = [(0, 128), (128, IS - 128)]
            ssilus = {}

            def shared_gate(mi):
                m0, msz = mchunks[mi]
                gs_ps = ps_s.tile([128, T], F32, name="gs_ps", tag="ps_s")
                for k in range(HK):
                    nc.tensor.matmul(gs_ps[0:msz, :],
                                     sgw_sb[:, k * IS + m0: k * IS + m0 + msz],
                                     xt_slice(k), start=k == 0, stop=k == HK - 1)
                ssilu_sb = actpool.tile([128, T], F16, name="ssilu_sb",
                                        tag="ssilu")
                nc.scalar.activation(ssilu_sb[0:msz, :], gs_ps[0:msz, :],
                                     AF.Silu)
                ssilus[mi] = ssilu_sb

            def shared_up(mi):
                m0, msz = mchunks[mi]
                us_ps = ps_s.tile([128, T], F32, name="us_ps", tag="ps_s")
                for k in range(HK):
                    nc.tensor.matmul(us_ps[0:msz, :],
                                     suw_sb[:, k * IS + m0: k * IS + m0 + msz],
                                     xt_slice(k), start=k == 0, stop=k == HK - 1)
                nc.vector.tensor_mul(gus_sb[0:msz, bass.ts(mi, T)],
                                     ssilus[mi][0:msz, :], us_ps[0:msz, :])

            def shared_down(t):
                ysb = ypool.tile([128, H], F16, name="ysb", tag="y")
                for n in range(H // 512):
                    y_ps = ps_s.tile([128, 512], F32, name="ysd_ps", tag="ps_s")
                    nc.tensor.matmul(
                        y_ps, gus_sb[:, t * 128: t * 128 + 128],
                        sdw_sb[:, n * 512: n * 512 + 512],
                        start=True, stop=False)
                    nc.tensor.matmul(
                        y_ps, gus_sb[0:IS - 128, T + t * 128: T + t * 128 + 128],
                        sdw_sb[0:IS - 128, H + n * 512: H + n * 512 + 512],
                        start=False, stop=True)
                    nc.vector.tensor_copy(ysb[:, bass.ts(n, 512)], y_ps)
                nc.scalar.dma_start(out=ys[bass.ts(t, 128), :], in_=ysb)

            def gate_phase(e):
                wga = wpool.tile([128, HH * I], F8, name="wga", tag="w")
                wdma(wga, wg[e, 0])
                wgb = wpool.tile([128, HH * I], F8, name="wgb", tag="w")
                wdma(wgb, wg[e, 1])
                silus = []
                for it in range(IC):
                    g_ps = ps_g.tile([128, C], F32, name="g_ps", tag="ps_g")
                    for k in range(HK):
                        w_sb = wga if k < HH else wgb
                        off = (k % HH) * I + it * 128
                        nc.tensor.matmul(g_ps, w_sb[:, off:off + 128],
                                         xg_sb[e][:, bass.ts(k, C)],
                                         start=k == 0, stop=k == HK - 1)
                    silu_sb = silupool.tile([128, C], F16, name="silu_sb",
                                            tag="silu")
                    nc.scalar.activation(silu_sb, g_ps, AF.Silu, scale=1.0 / WS)
                    silus.append(silu_sb)
                return silus

            def up_phase(e, silus):
                wua = wpool.tile([128, HH * I], F8, name="wua", tag="w")
                wdma(wua, wu[e, 0])
                wub = wpool.tile([128, HH * I], F8, name="wub", tag="w")
                wdma(wub, wu[e, 1])
                gu_sb = gupool.tile([128, IC * C], F16, name="gu_sb", tag="gu")
                for it in range(IC):
                    u_ps = ps_u.tile([128, C], F32, name="u_ps", tag="ps_u")
                    for k in range(HK):
                        w_sb = wua if k < HH else wub
                        off = (k % HH) * I + it * 128
                        nc.tensor.matmul(u_ps, w_sb[:, off:off + 128],
                                         xg_sb[e][:, bass.ts(k, C)],
                                         start=k == 0, stop=k == HK - 1)
                    nc.vector.tensor_mul(gu_sb[:, bass.ts(it, C)],
                                         silus[it], u_ps)
                return gu_sb

            def down_dma(e):
                tiles = []
                for n in range(ND):
                    wd_sb = wdpool.tile([128, IC * 512], F8, name="wd_sb",
                                        tag="wd")
                    wdma(wd_sb, wd[e, n])
                    tiles.append(wd_sb)
                return tiles

            def down_phase(e, gu_sb, wd_tiles):
                y_sb = ypool.tile([128, HK * C], F16, name="y_sb", tag="y")
                for n in range(ND):
                    wd_sb = wd_tiles[n]
                    for hh in range(4):
                        ht = n * 4 + hh
                        y_ps = ps_y.tile([128, C], F32, name="y_ps", tag="ps_y")
                        for it in range(IC):
                            lhs = wd_sb[:, it * 512 + hh * 128:
                                        it * 512 + hh * 128 + 128]
                            nc.tensor.matmul(y_ps, lhs,
                                             gu_sb[:, bass.ts(it, C)],
                                             start=it == 0, stop=it == IC - 1)
                        nc.vector.tensor_scalar_mul(
                            y_sb[:, bass.ts(ht, C)], y_ps, 1.0 / (WS * WS))
                nc.scalar.dma_start(out=yr[e], in_=y_sb)

            # --- emission schedule: shared work fills expert DMA stalls
            shared_gate(0)                  # PE start: needs sgw + xt only
            silus0 = gate_phase(0)
            shared_gate(1)
            gu0 = up_phase(0, silus0)
            nc.scalar.dma_start(out=suw_sb, in_=suw)
            shared_up(0)
            wd0 = down_dma(0)
            nc.scalar.dma_start(out=sdw_sb[:, 0:H], in_=sdw[0:128, :])
            nc.scalar.dma_start(out=sdw_sb[0:IS - 128, H:2 * H], in_=sdw[128:IS, :])
            down_phase(0, gu0, wd0)
            shared_up(1)
            silus1 = gate_phase(1)
            gu1 = up_phase(1, silus1)
            shared_down(0)
            wd1 = down_dma(1)
            shared_down(1)
            down_phase(1, gu1, wd1)
            silus2 = gate_phase(2)
            gu2 = up_phase(2, silus2)
            shared_down(2)
            wd2 = down_dma(2)
            shared_down(3)
            down_phase(2, gu2, wd2)
            silus3 = gate_phase(3)
            gu3 = up_phase(3, silus3)
            wd3 = down_dma(3)
            down_phase(3, gu3, wd3)

    if compile:
        nc.compile()
    return nc


def _swizzle(a: np.ndarray, nchunk: int) -> np.ndarray:
    """[nchunk*128, F] -> [128, nchunk*F] with partition = inner 128."""
    n, f = a.shape
    assert n == nchunk * 128
    return np.ascontiguousarray(
        a.reshape(nchunk, 128, f).transpose(1, 0, 2).reshape(128, nchunk * f))


def _quant_fp8(w: np.ndarray) -> np.ndarray:
    w = np.asarray(w, np.float32) * WS
    np.clip(w, -FP8_MAX, FP8_MAX, out=w)
    return w.astype(ml_dtypes.float8_e3m4)


def _prepare(x, router_w, router_b, gate_w, up_w, down_w,
             shared_gate_w, shared_up_w, shared_down_w):
    f16 = np.float16
    xf = np.asarray(x, np.float32).reshape(T, H)

    # --- routing on host in float64 (matches reference ranking) ---
    logits = xf.astype(np.float64) @ np.asarray(router_w, np.float64).T \
        + np.asarray(router_b, np.float64)
    top_idx = np.argsort(-logits, axis=1, kind="stable")[:, :K]
    top_vals = np.take_along_axis(logits, top_idx, axis=1)
    ex = np.exp(top_vals - top_vals.max(axis=1, keepdims=True))
    rw = (ex / ex.sum(axis=1, keepdims=True) * SCALE).astype(np.float32)

    tok_of = [np.where((top_idx == e).any(axis=1))[0] for e in range(E)]
    w_of = []
    for e in range(E):
        sel = top_idx[tok_of[e]] == e
        w_of.append((rw[tok_of[e]] * sel).sum(axis=1).astype(np.float32))
    max_n = max(len(t) for t in tok_of)
    C = max(32, ((max_n + 31) // 32) * 32)

    x16 = xf.astype(f16)
    xt_np = _swizzle(np.ascontiguousarray(x16.T), HK) \
        .reshape(128, 2, HH * T).transpose(1, 0, 2).copy()   # [2, 128, HH*T]

    gate_w = np.asarray(gate_w, np.float32)
    up_w = np.asarray(up_w, np.float32)
    down_w = np.asarray(down_w, np.float32)

    in_maps = []
    for c in range(NCORES):
        m = {}
        xg_np = np.zeros((EL, 128, HK * C), f16)
        wg_np = np.empty((EL, 2, 128, HH * I), ml_dtypes.float8_e3m4)
        wu_np = np.empty((EL, 2, 128, HH * I), ml_dtypes.float8_e3m4)
        wd_np = np.empty((EL, ND, 128, IC * 512), ml_dtypes.float8_e3m4)
        for le in range(EL):
            e = c * EL + le
            idx = tok_of[e]
            if len(idx):
                xe = np.zeros((H, C), f16)
                xe[:, :len(idx)] = x16[idx].T
                xg_np[le] = _swizzle(xe, HK)
            qg = _quant_fp8(gate_w[e].T)                    # [H, I]
            qu = _quant_fp8(up_w[e].T)
            for hh in range(2):
                sl = slice(hh * HH * 128, (hh + 1) * HH * 128)
                wg_np[le, hh] = _swizzle(qg[sl], HH)
                wu_np[le, hh] = _swizzle(qu[sl], HH)
            qd = _quant_fp8(down_w[e].T)                    # [I, H]
            for n in range(ND):
                wd_np[le, n] = _swizzle(
                    np.ascontiguousarray(qd[:, n * 512:(n + 1) * 512]), IC)
        m["xg"], m["wg"], m["wu"], m["wd"] = xg_np, wg_np, wu_np, wd_np
        isl = slice(c * IS, (c + 1) * IS)
        m["sgw"] = _swizzle(
            np.ascontiguousarray(shared_gate_w[isl].T).astype(f16), HK)
        m["suw"] = _swizzle(
            np.ascontiguousarray(shared_up_w[isl].T).astype(f16), HK)
        m["sdw"] = np.ascontiguousarray(shared_down_w[:, isl].T).astype(f16)
        m["xt"] = xt_np
        in_maps.append(m)
    return in_maps, tok_of, w_of, C


def _install_ntff_hook():
    import types
    if "antenv.axon_hooks" in sys.modules:
        return
    try:
        import antenv  # noqa: F401
        from trn_agent_boot.trn_boot import _ntff_profile_via_ctypes
        hook = _ntff_profile_via_ctypes("/opt/axon/libaxon_pjrt.so")
    except Exception:
        return
    mod = types.ModuleType("antenv.axon_hooks")
    mod._hook = hook
    mod.get_axon_ntff_profile_hook = lambda: mod._hook

    def _set(h):
        mod._hook = h

    mod.set_axon_ntff_profile_hook = _set
    sys.modules["antenv.axon_hooks"] = mod


def kernel(x, router_w, router_b, gate_w, up_w, down_w,
           shared_gate_w, shared_up_w, shared_down_w):
    in_maps, tok_of, w_of, C = _prepare(
        x, router_w, router_b, gate_w, up_w, down_w,
        shared_gate_w, shared_up_w, shared_down_w)

    nc = _build_program(C)
    trace = os.environ.get("MOE_KERNEL_TRACE", "0") == "1"
    if trace:
        _install_ntff_hook()
    res = bass_utils.run_bass_kernel_spmd(
        nc, in_maps, core_ids=list(range(NCORES)), trace=trace)
    kernel.last_results = res

    out = np.zeros((T, H), np.float32)
    for c in range(NCORES):
        out += res.results[c]["ys"]
        for le in range(EL):
            e = c * EL + le
            idx = tok_of[e]
            if len(idx):
                y = res.results[c]["yr"][le].reshape(128, HK, C) \
                    .transpose(1, 0, 2).reshape(H, C)
                out[idx] += w_of[e][:, None] * \
                    y[:, :len(idx)].T.astype(np.float32)
    return out.reshape(B, S, H).astype(np.float32)
